# revision 28
# baseline (speedup 1.0000x reference)
"""Trainium2 Bass kernel for nn_MixtureAttention.

Math: the reference builds a (c,c) pairwise Cauchy-product matrix per batch,
row-normalizes it, and keeps only the diagonal.  With
    u_d[c,p] = (mu[p,d] - mu[c,d]) / sig[c,d]
the kept diagonal reduces to
    coef[c] = 1 / sum_p prod_d 1/(1 + u_d[c,p]^2)
(`pi` cancels in the row normalization), and y[b,ch,c] = x[b,ch] * coef[b,c].

Sharding: 8 cores; core k handles batch k//2, c-rows [ (k%2)*2048, +2048 ).
Each core computes its 2048x4096 pairwise block fully on-chip:
  - ACT: u^2 via Square activation with per-partition scale/bias (+ one "+1")
  - GPSIMD: one "+1"
  - DVE: two fused affine-mul ops, two fast reciprocals, one fused
    mul+row-reduce, per (128-row, 2048-point) tile
  - PE: final outer product x (x) coef
"""

import numpy as np

B, C, D, CH = 4, 4096, 4, 256
NCORES = 8
CW = C // 2            # 2048 c-rows per core (2 cores per batch)
NBLK = CW // 128       # 16 row blocks
PCH = 2048             # p-chunk size
NPCH = C // PCH        # 2
NOUT = 512             # matmul free-dim tile for the output outer product

_cache = {}


def _get_pp1():
    """Register a custom DVE op: out = (in0 + s0) * (in1 + s1).

    Fuses the '+1' pre-add into the pair product, saving one DVE pass per
    tile. Registered into concourse's op table at runtime; uop shas are
    self-pinned by compiling once and reading the reported digest.
    """
    if "pp1" in _cache:
        return _cache["pp1"]
    import re

    from concourse import dve_ops as DO
    from concourse.dve_spec import C0, C1, Spec, Src0, Src1

    name = "PROD_PLUS1_ANT"
    spec = Spec(
        body=(Src0 + C0) * (Src1 + C1),
        reference=lambda in0, in1, c0, c1, c2: (in0 + c0) * (in1 + c1),
    )
    shas = {}
    for ver in ("v3", "v4"):
        probe = DO.DveOp(name + "_PROBE", spec, subdim=False, uops_sha={})
        if name + "_PROBE" not in DO._SUB_OPCODE_FOR_NAME:
            DO._SUB_OPCODE_FOR_NAME[name + "_PROBE"] = 0x1F
        try:
            probe.compile(ver)
        except ValueError as e:
            m = re.search(r'"(?:v3|v4)"\]="([0-9a-f]+)"', str(e))
            if not m:
                raise
            shas[ver] = m.group(1)
    op = DO.DveOp(name, spec, subdim=False, uops_sha=shas)
    if name not in DO._SUB_OPCODE_FOR_NAME:
        DO.OPS.append(op)
        DO._SUB_OPCODE_FOR_NAME[name] = DO._CUSTOM_DVE_ROW_BASE + len(DO.OPS) - 1
        assert DO._SUB_OPCODE_FOR_NAME[name] < 0x20
    DO.CUSTOM_DVE_SPECS[name] = spec
    _cache["pp1"] = op
    return op


def _build(bench_nrep=None, bench_span="main"):
    import concourse.bacc as bacc
    import concourse.mybir as mybir
    from concourse.tile import TileContext

    f32 = mybir.dt.float32
    Alu = mybir.AluOpType
    Act = mybir.ActivationFunctionType

    pp1 = _get_pp1()
    nc = bacc.Bacc(None, target_bir_lowering=False)
    ptsT = nc.declare_dram_parameter("ptsT", [D, C], f32, isOutput=False)
    isg_r = nc.declare_dram_parameter("isg_r", [128, NBLK * D], f32, isOutput=False)
    nbs_r = nc.declare_dram_parameter("nbs_r", [128, NBLK * D], f32, isOutput=False)
    xv = nc.declare_dram_parameter("xv", [1, CH], f32, isOutput=False)
    y = nc.declare_dram_parameter("y", [CH, CW], f32, isOutput=True)

    with TileContext(nc) as tc:
        with (
            tc.tile_pool(name="persist", bufs=1) as pp,
            tc.tile_pool(name="bpool", bufs=1) as bp,
            tc.tile_pool(name="work", bufs=1) as wp,
            tc.tile_pool(name="psum", bufs=4, space="PSUM") as psp,
            tc.tile_pool(name="dram", bufs=1, space="DRAM") as dp,
        ):
            scr = dp.tile([128 * NBLK], f32, name="scr")
            inv_sg = pp.tile([128, NBLK, D], f32)
            nc.sync.dma_start(
                out=inv_sg[:, :, :], in_=isg_r.rearrange("p (n d) -> p n d", d=D)
            )
            nbias = pp.tile([128, NBLK, D], f32)
            nc.sync.dma_start(
                out=nbias[:, :, :], in_=nbs_r.rearrange("p (n d) -> p n d", d=D)
            )
            xv_sb = pp.tile([1, CH], f32)
            nc.sync.dma_start(out=xv_sb[0:1, :], in_=xv[0:1, :])

            Racc = pp.tile([128, NBLK, NPCH], f32)
            junkacc = pp.tile([128, 2], f32)

            Bt = [bp.tile([128, C], f32, name=f"bt{dd}") for dd in range(D)]

            def bcast_loop():
                hp = PCH // 2
                for jj in range(2 * NPCH):
                    for dd in range(D):
                        nc.sync.dma_start(
                            out=Bt[dd][:, jj * hp : (jj + 1) * hp],
                            in_=ptsT[dd : dd + 1, jj * hp : (jj + 1) * hp].broadcast_to(
                                [128, hp]
                            ),
                        )

            def main_loop(n_lo, n_hi):
              for n in range(n_lo, n_hi):
                for j in range(NPCH):
                    sq = []
                    for dd in range(D):
                        s = wp.tile([128, PCH], f32, tag="sq", bufs=6, name="sq")
                        nc.scalar.activation(
                            s[:, :],
                            Bt[dd][:, j * PCH : (j + 1) * PCH],
                            Act.Square,
                            bias=nbias[:, n, dd : dd + 1],
                            scale=inv_sg[:, n, dd : dd + 1],
                        )
                        sq.append(s)
                    # chain: Q = ((1+sq0)(1+sq1))(1+sq2))(1+sq3); first pair fused
                    q1 = wp.tile([128, PCH], f32, tag="q", bufs=4, name="q1")
                    nc.vector._custom_dve(
                        pp1, out=q1[:, :], in0=sq[0][:, :], in1=sq[1][:, :],
                        s0=1.0, s1=1.0,
                    )
                    q2 = wp.tile([128, PCH], f32, tag="q", bufs=4, name="q2")
                    nc.vector.affine_mul_reduce(
                        out=q2[:, :], accum_out=junkacc[:, 1:2],
                        in0=sq[2][:, :], in1=q1[:, :], scale=1.0, bias=1.0,
                    )
                    q3 = wp.tile([128, PCH], f32, tag="q", bufs=4, name="q3")
                    nc.vector.affine_mul_reduce(
                        out=q3[:, :], accum_out=junkacc[:, 0:1],
                        in0=sq[3][:, :], in1=q2[:, :], scale=1.0, bias=1.0,
                    )
                    r = wp.tile([128, PCH], f32, tag="r", bufs=2, name="r")
                    nc.vector.reciprocal_approx_fast(out=r[:, :], in_=q3[:, :])
                    junk = wp.tile([128, PCH], f32, tag="junk", bufs=2, name="junk")
                    nc.vector.tensor_scalar(
                        junk[:, :], r[:, :], 0.0, None, Alu.add, Alu.add,
                        accum_out=Racc[:, n, j : j + 1],
                    )
                    # warm the PE p-state shortly before each half's matmuls
                    if n % (NBLK // 2) >= NBLK // 2 - 2:
                        psd = psp.tile([128, NOUT], f32, tag="ps", name="psd")
                        nc.tensor.matmul(
                            psd[:, :],
                            xv_sb[0:1, 0:128],
                            Bt[0][0:1, 0:NOUT],
                            start=True,
                            stop=True,
                        )

            HB = NBLK // 2          # blocks per epilogue half
            HC = HB * 128           # c-columns per half

            def epilogue(half):
                nsl = slice(half * HB, (half + 1) * HB)
                Rsum = pp.tile([128, HB], f32, name="Rsum", tag="Rsum", bufs=2)
                nc.vector.tensor_tensor(
                    Rsum[:, :], Racc[:, nsl, 0], Racc[:, nsl, 1], Alu.add
                )
                coef = pp.tile([128, HB], f32, name="coef", tag="coef", bufs=2)
                nc.vector.reciprocal(coef[:, :], Rsum[:, :])

                # transpose coef (128, HB) -> row (1, HC) via a DRAM bounce
                nc.sync.dma_start(
                    out=scr.rearrange("(p n) -> p n", p=128)[:, nsl], in_=coef[:, :]
                )
                crow = pp.tile([1, HC], f32, name="crow", tag="crow", bufs=2)
                nc.sync.dma_start(
                    out=crow[0:1, :].rearrange("a (n p) -> a n p", n=HB),
                    in_=scr.rearrange("(p n) -> n p", n=NBLK)[nsl, :],
                )

                # y[ch, c] = x[ch] * coef[c] as K=1 outer-product matmuls
                for h in range(CH // 128):
                    for qk in range(HC // NOUT):
                        ps = psp.tile([128, NOUT], f32, tag="ps", name="ps")
                        nc.tensor.matmul(
                            ps[:, :],
                            xv_sb[0:1, h * 128 : (h + 1) * 128],
                            crow[0:1, qk * NOUT : (qk + 1) * NOUT],
                            start=True,
                            stop=True,
                        )
                        ysb = wp.tile([128, NOUT], f32, tag="ysb", bufs=2, name="ysb")
                        nc.scalar.copy(ysb[:, :], ps[:, :])
                        nc.sync.dma_start(
                            out=y[
                                h * 128 : (h + 1) * 128,
                                half * HC + qk * NOUT : half * HC + (qk + 1) * NOUT,
                            ],
                            in_=ysb[:, :],
                        )

            def whole():
                bcast_loop()
                main_loop(0, NBLK // 2)
                epilogue(0)
                main_loop(NBLK // 2, NBLK)
                epilogue(1)

            if bench_nrep is None:
                whole()
            elif bench_span == "main":
                bcast_loop()
                with tc.For_i(0, bench_nrep, 1):
                    main_loop(0, NBLK)
                epilogue(0)
                epilogue(1)
            elif bench_span == "bcast":
                with tc.For_i(0, bench_nrep, 1):
                    bcast_loop()
                main_loop(0, NBLK)
                epilogue(0)
                epilogue(1)
            elif bench_span == "epi":
                bcast_loop()
                main_loop(0, NBLK)
                with tc.For_i(0, bench_nrep, 1):
                    epilogue(0)
                    epilogue(1)
            else:
                import concourse.mybir as _mb

                with tc.For_i(
                    0, bench_nrep, 1,
                    staggered_reset=True,
                    hint_engines=(_mb.EngineType.DVE, _mb.EngineType.Activation),
                ):
                    whole()
    nc.finalize()
    return nc


def _get_nc():
    if "nc" not in _cache:
        _cache["nc"] = _build()
    return _cache["nc"]


def _in_maps(x, mu, sig):
    maps = []
    for k in range(NCORES):
        b = k // 2
        half = k % 2
        sl = slice(half * CW, (half + 1) * CW)
        mu_b = np.asarray(mu[b], dtype=np.float32)
        sig_c = np.asarray(sig[b, sl], dtype=np.float32)
        inv = (np.float32(1.0) / sig_c).astype(np.float32)
        nbs = (-(mu_b[sl] * inv)).astype(np.float32)

        def _rearr(a):
            return np.ascontiguousarray(
                a.reshape(NBLK, 128, D).transpose(1, 0, 2).reshape(128, -1)
            )

        maps.append(
            {
                "ptsT": np.ascontiguousarray(mu_b.T),
                "isg_r": _rearr(inv),
                "nbs_r": _rearr(nbs),
                "xv": np.ascontiguousarray(
                    np.asarray(x[b, :, 0], dtype=np.float32)[None, :]
                ),
            }
        )
    return maps


def kernel(x, pi, mu, sig):
    from concourse.bass_utils import run_bass_kernel_spmd

    nc = _get_nc()
    res = run_bass_kernel_spmd(nc, _in_maps(x, mu, sig), list(range(NCORES))).results
    y = np.empty((B, CH, C), np.float32)
    for k in range(NCORES):
        b = k // 2
        half = k % 2
        y[b, :, half * CW : (half + 1) * CW] = res[k]["y"]
    return y
